# revision 1
# baseline (speedup 1.0000x reference)
"""Causal single-head attention (S=4096, D=1024, fp32) on 8 TRN2 NeuronCores.

v6 (pair-split proj + chunked pair-AllGather + SBUF-accumulated A@V) with the
serialization fixed: attention quarters are emitted INSIDE the projection
stream (attn t0 between proj q2 and q3, the rest after), so the PE consumes
gathered chunks as they land instead of finishing all projections first.
Projection accumulation and score matmuls share one PSUM pool (4 banks) so
the total PSUM stays at 8: 4 shared + 3 A@V scratch + 1 packed sums.
"""

import numpy as np
import ml_dtypes

import concourse.bacc as bacc
import concourse.tile as tile
from concourse import mybir
from concourse.bass_utils import run_bass_kernel_spmd

S = 4096
D = 1024
NCORES = 8
ROWS = 512
P = 128
DC = 8
OT = 8
HALF = 2048
NQT = 4
NJT = 32
BF = mybir.dt.bfloat16
F32 = mybir.dt.float32
EXP = mybir.ActivationFunctionType.Exp
PAIRS = [[0, 1], [2, 3], [4, 5], [6, 7]]

bf16 = ml_dtypes.bfloat16


def build_nc():
    nc = bacc.Bacc(None, target_bir_lowering=False, debug=False)

    xq = nc.declare_dram_parameter("xqt", [D, ROWS], BF, isOutput=False)
    xk = nc.declare_dram_parameter("xkh", [D, HALF], BF, isOutput=False)
    xv = nc.declare_dram_parameter("xvh", [D, HALF], BF, isOutput=False)
    wq = nc.declare_dram_parameter("wqt", [D, D], BF, isOutput=False)
    wk = nc.declare_dram_parameter("wkt", [D, D], BF, isOutput=False)
    wv = nc.declare_dram_parameter("wvt", [D, D], BF, isOutput=False)
    msk = nc.declare_dram_parameter("mask01", [NJT, P, ROWS], BF, isOutput=False)
    out = nc.declare_dram_parameter("out", [ROWS, D], F32, isOutput=True)

    kvin = [nc.dram_tensor(f"kvin{t}", [P, 16, 512], BF) for t in range(NQT)]
    kvout = [nc.dram_tensor(f"kvout{t}", [2 * P, 16, 512], BF) for t in range(NQT)]


    with tile.TileContext(nc) as tc:
        with (
            tc.tile_pool(name="persist", bufs=1) as persist,
            tc.tile_pool(name="proj", bufs=1) as kp,
            tc.tile_pool(name="stg", bufs=6) as stg,
            tc.tile_pool(name="xs", bufs=2) as xs,
            tc.tile_pool(name="kvs", bufs=2) as kvs,
            tc.tile_pool(name="att", bufs=6) as ap,
            tc.tile_pool(name="att_out", bufs=3) as op,
            tc.tile_pool(name="pps", bufs=5, space="PSUM") as pps,
            tc.tile_pool(name="avs", bufs=2, space="PSUM") as avsum,
            tc.tile_pool(name="ops", bufs=1, space="PSUM") as opsum,
        ):
            ones = persist.tile([P, 16], BF, tag="ones", name="ones")
            nc.vector.memset(ones[:], 1.0)
            zbias = persist.tile([P, 1], F32, tag="zbias", name="zbias")
            nc.vector.memset(zbias[:], 0.0)
            qT = [persist.tile([P, ROWS], BF, tag=f"qT{t}", name=f"qT{t}") for t in range(OT)]
            acc = {}
            for isub in range(4):
                for ob in range(2):
                    acc[isub, ob] = persist.tile([P, 512], F32, tag=f"acc{isub}{ob}", name=f"acc{isub}{ob}")
                    nc.vector.memset(acc[isub, ob][:], 0.0)
            sums_bank = opsum.tile([P, 64], F32, tag="sums", name="sums")

            wk_t = [kp.tile([P, D], BF, tag=f"wk{d_}", name=f"wk{d_}") for d_ in range(DC)]
            wv_t = [kp.tile([P, D], BF, tag=f"wv{d_}", name=f"wv{d_}") for d_ in range(DC)]
            xk_t = {}
            xv_t = {}

            def load_x_quarter(t, k_first=False):
                for d_ in range(DC):
                    xk_t[t, d_] = xs.tile([P, 512], BF, tag=f"xk{d_}", name=f"xk{d_}")
                    nc.sync.dma_start(out=xk_t[t, d_][:], in_=xk[d_ * P:(d_ + 1) * P, t * 512:(t + 1) * 512])
                    if not k_first:
                        xv_t[t, d_] = xs.tile([P, 512], BF, tag=f"xv{d_}", name=f"xv{d_}")
                        nc.sync.dma_start(out=xv_t[t, d_][:], in_=xv[d_ * P:(d_ + 1) * P, t * 512:(t + 1) * 512])
                if k_first:
                    for d_ in range(DC):
                        xv_t[t, d_] = xs.tile([P, 512], BF, tag=f"xv{d_}", name=f"xv{d_}")
                        nc.sync.dma_start(out=xv_t[t, d_][:], in_=xv[d_ * P:(d_ + 1) * P, t * 512:(t + 1) * 512])

            def kv_quarter(t):
                for ohi in range(OT):
                    ps = pps.tile([P, 512], F32, tag="pp", name="ppk")
                    for d_ in range(DC):
                        nc.tensor.matmul(
                            ps[:],
                            lhsT=wk_t[d_][:, ohi * P:(ohi + 1) * P],
                            rhs=xk_t[t, d_][:],
                            start=(d_ == 0),
                            stop=(d_ == DC - 1),
                        )
                    sg = stg.tile([P, 512], BF, tag="sg", name="sg")
                    nc.scalar.copy(sg[:], ps[:])
                    nc.gpsimd.dma_start(out=kvin[t][:, ohi, :], in_=sg[:])
                for jh in range(4):
                    for ob in range(2):
                        ps = pps.tile([P, 512], F32, tag="pp", name="ppv")
                        for d_ in range(DC):
                            nc.tensor.matmul(
                                ps[:],
                                lhsT=xv_t[t, d_][:, jh * P:(jh + 1) * P],
                                rhs=wv_t[d_][:, ob * 512:(ob + 1) * 512],
                                start=(d_ == 0),
                                stop=(d_ == DC - 1),
                            )
                        sg = stg.tile([P, 512], BF, tag="sg", name="sg")
                        nc.scalar.copy(sg[:], ps[:])
                        nc.gpsimd.dma_start(out=kvin[t][:, 8 + ob * 4 + jh, :], in_=sg[:])
                nc.gpsimd.collective_compute(
                    "AllGather",
                    mybir.AluOpType.bypass,
                    replica_groups=PAIRS,
                    ins=[kvin[t][:].opt()],
                    outs=[kvout[t][:].opt()],
                )

            def attn_quarter(qtr):
                t, g = qtr // 2, qtr % 2
                ktq = kvs.tile([P, OT, 512], BF, tag="ktq", name="ktq")
                nc.scalar.dma_start(out=ktq[:], in_=kvout[t][g * P:(g + 1) * P, 0:8, :])
                vtq = kvs.tile([P, OT, 512], BF, tag="vtq", name="vtq")
                nc.scalar.dma_start(out=vtq[:], in_=kvout[t][g * P:(g + 1) * P, 8:16, :])
                ptq = []
                for jl in range(4):
                    jt = qtr * 4 + jl
                    sp = pps.tile([P, ROWS], F32, tag="pp", name="sps")
                    for oc in range(OT):
                        nc.tensor.matmul(
                            sp[:],
                            lhsT=ktq[:, oc, jl * P:(jl + 1) * P],
                            rhs=qT[oc][:],
                            start=(oc == 0),
                            stop=(oc == OT - 1),
                        )
                    pt = ap.tile([P, ROWS], BF, tag="pt", name="pt")
                    nc.scalar.activation(pt[:], sp[:], EXP, bias=zbias[:])
                    mt = ap.tile([P, ROWS], BF, tag="mt", name="mt")
                    nc.gpsimd.dma_start(out=mt[:], in_=msk[jt, :, :])
                    nc.vector.tensor_mul(pt[:], pt[:], mt[:])
                    ptq.append(pt)
                    for isub in range(4):
                        nc.tensor.matmul(
                            sums_bank[:, isub * 16:(isub + 1) * 16],
                            lhsT=pt[:, isub * P:(isub + 1) * P],
                            rhs=ones[:],
                            start=(jt == 0 and isub == 0),
                            stop=(jt == NJT - 1 and isub == 3),
                            skip_group_check=True,
                        )
                for isub in range(4):
                    for ob in range(2):
                        sc = avsum.tile([P, 512], F32, tag="avs", name="avs")
                        for jl in range(4):
                            nc.tensor.matmul(
                                sc[:],
                                lhsT=ptq[jl][:, isub * P:(isub + 1) * P],
                                rhs=vtq[:, ob * 4 + jl, :],
                                start=(jl == 0),
                                stop=(jl == 3),
                            )
                        nc.vector.tensor_add(acc[isub, ob][:], acc[isub, ob][:], sc[:])

            # ---- interleaved schedule: K weights + x_k q0 load first ----
            for d_ in range(DC):
                nc.sync.dma_start(out=wk_t[d_][:], in_=wk[d_ * P:(d_ + 1) * P, :])
            load_x_quarter(0, k_first=True)
            for d_ in range(DC):
                nc.sync.dma_start(out=wv_t[d_][:], in_=wv[d_ * P:(d_ + 1) * P, :])
            kv_quarter(0)
            load_x_quarter(1)
            kv_quarter(1)

            xq_t = [kp.tile([P, ROWS], BF, tag=f"xq{d_}", name=f"xq{d_}") for d_ in range(DC)]
            wq_t = [kp.tile([P, D], BF, tag=f"wq{d_}", name=f"wq{d_}") for d_ in range(DC)]
            for d_ in range(DC):
                nc.sync.dma_start(out=xq_t[d_][:], in_=xq[d_ * P:(d_ + 1) * P, :])
                nc.sync.dma_start(out=wq_t[d_][:], in_=wq[d_ * P:(d_ + 1) * P, :])
            for t in range(OT):
                ps = pps.tile([P, ROWS], F32, tag="pp", name="ppq")
                for d_ in range(DC):
                    nc.tensor.matmul(
                        ps[:],
                        lhsT=wq_t[d_][:, t * P:(t + 1) * P],
                        rhs=xq_t[d_][:],
                        start=(d_ == 0),
                        stop=(d_ == DC - 1),
                    )
                nc.vector.tensor_copy(qT[t][:], ps[:])

            load_x_quarter(2)
            kv_quarter(2)
            attn_quarter(0)
            load_x_quarter(3)
            kv_quarter(3)
            for qtr in range(1, 8):
                attn_quarter(qtr)

            for isub in range(4):
                ssb = op.tile([P, 1], F32, tag="ssb", name="ssb")
                nc.vector.tensor_copy(ssb[:], sums_bank[:, isub * 16:isub * 16 + 1])
                rec = op.tile([P, 1], F32, tag=f"rec{isub}", name=f"rec{isub}")
                nc.vector.reciprocal(rec[:], ssb[:])
                for ob in range(2):
                    osb = op.tile([P, 512], F32, tag="osb", name="osb")
                    nc.vector.tensor_scalar_mul(osb[:], acc[isub, ob][:], rec[:])
                    nc.sync.dma_start(out=out[isub * P:(isub + 1) * P, ob * 512:(ob + 1) * 512], in_=osb[:])
    return nc


_CACHE = {}


def _get_nc():
    if "nc" not in _CACHE:
        nc = build_nc()
        nc.compile()
        _CACHE["nc"] = nc
    return _CACHE["nc"]


def build_in_maps(inputs):
    x_q = np.asarray(inputs["encodings_for_q"], dtype=np.float32)
    x_k = np.asarray(inputs["encodings_for_k"], dtype=np.float32)
    x_v = np.asarray(inputs["encodings_for_v"], dtype=np.float32)
    W_q = np.asarray(inputs["W_q"], dtype=np.float32)
    W_k = np.asarray(inputs["W_k"], dtype=np.float32)
    W_v = np.asarray(inputs["W_v"], dtype=np.float32)

    wqt = np.ascontiguousarray(W_q.T).astype(bf16)
    wkt = np.ascontiguousarray(W_k.T / np.sqrt(D)).astype(bf16)
    wvt = np.ascontiguousarray(W_v.T).astype(bf16)

    causal = (np.arange(S)[:, None] <= np.arange(S)[None, :])

    in_maps = []
    for c in range(NCORES):
        rows = slice(ROWS * c, ROWS * (c + 1))
        h = slice(HALF * (c % 2), HALF * (c % 2 + 1))
        xqt_c = np.ascontiguousarray(x_q[rows].T).astype(bf16)
        xkh_c = np.ascontiguousarray(x_k[h].T).astype(bf16)
        xvh_c = np.ascontiguousarray(x_v[h].T).astype(bf16)
        m = causal[:, rows]
        mg = m.reshape(NJT, P, ROWS)
        order = []
        for qtr in range(8):
            t, g = qtr // 2, qtr % 2
            for jl in range(4):
                order.append(16 * g + 4 * t + jl)
        mask_c = np.ascontiguousarray(mg[order]).astype(bf16)
        in_maps.append(
            dict(
                xqt=xqt_c, xkh=xkh_c, xvh=xvh_c,
                wqt=wqt, wkt=wkt, wvt=wvt,
                mask01=mask_c,
            )
        )
    return in_maps


def kernel(**inputs):
    nc = _get_nc()
    in_maps = build_in_maps(inputs)
    res = run_bass_kernel_spmd(nc, in_maps, list(range(NCORES)))
    outs = [np.asarray(res.results[i]["out"], dtype=np.float32) for i in range(NCORES)]
    return np.concatenate(outs, axis=0)



# revision 2
# speedup vs baseline: 1.4595x; 1.4595x over previous
"""Causal single-head attention (S=4096, D=1024, fp32) on 8 TRN2 NeuronCores.

v7: uniform causal stripe decomposition (SPMD-friendly).
- Rows are 16-row stripes dealt round-robin to cores (stripe s -> core s%8),
  packed per-core in DESCENDING row order. For col-tile j (128 cols) the live
  rows are then exactly the first N_j = 512-16*j packed rows on EVERY core, so
  one instruction schedule serves all cores; only a single 16-row diagonal
  stripe per tile needs masking, via one per-core [128,16] 0/1 mask input.
- K/V projections are 8-way sharded: core c projects positions [512c,512c+512)
  and two 8-way Shared-output AllGathers (position halves a/b, K^T+V packed)
  broadcast them. Scores / exp / row-sum / A@V stream over the gathered slabs.
- Row sums accumulate in one PSUM bank across all tiles (ones-matmul); A@V
  accumulates per chunk-half in PSUM then adds into SBUF fp32 accumulators.
"""

import numpy as np
import ml_dtypes

import concourse.bacc as bacc
import concourse.tile as tile
from concourse import mybir
from concourse.bass_utils import run_bass_kernel_spmd

S = 4096
D = 1024
NCORES = 8
ROWS = 512
P = 128
DC = 8
BF = mybir.dt.bfloat16
F32 = mybir.dt.float32
EXP = mybir.ActivationFunctionType.Exp
ALL8 = [[0, 1, 2, 3, 4, 5, 6, 7]]

bf16 = ml_dtypes.bfloat16


def build_nc():
    nc = bacc.Bacc(None, target_bir_lowering=False, debug=False)

    xq = nc.declare_dram_parameter("xqt", [D, ROWS], BF, isOutput=False)
    xk = nc.declare_dram_parameter("xkt", [D, ROWS], BF, isOutput=False)
    xv = nc.declare_dram_parameter("xvt", [D, ROWS], BF, isOutput=False)
    wq = nc.declare_dram_parameter("wqt", [D, D], BF, isOutput=False)
    wk = nc.declare_dram_parameter("wkt", [D, D], BF, isOutput=False)
    wv = nc.declare_dram_parameter("wvt", [D, D], BF, isOutput=False)
    msk = nc.declare_dram_parameter("mk", [P, 16], BF, isOutput=False)
    out = nc.declare_dram_parameter("out", [ROWS, D], F32, isOutput=True)

    # kvin layout [128, 4096] bf16 (1MB): [:, 256*oc : 256*oc+256] = K^T d-chunk
    # oc for this half's 256 positions; [:, 2048 + 1024*jl + 512*ob : +512] =
    # V rows for local position-tile jl (0/1) and output half ob.
    kvin = [nc.dram_tensor(f"kvin{h}", [P, 4096], BF) for h in range(2)]
    kvout = [
        nc.dram_tensor(f"kvout{h}", [NCORES * P, 4096], BF, addr_space="Shared")
        for h in range(2)
    ]

    with tile.TileContext(nc) as tc:
        with (
            tc.tile_pool(name="persist", bufs=1) as persist,
            tc.tile_pool(name="kp", bufs=1) as kp,
            tc.tile_pool(name="stg", bufs=6) as stg,
            tc.tile_pool(name="kvs", bufs=3) as kvs,
            tc.tile_pool(name="att", bufs=6) as ap,
            tc.tile_pool(name="att_out", bufs=4) as op,
            tc.tile_pool(name="pps", bufs=3, space="PSUM") as pps,
            tc.tile_pool(name="avs", bufs=3, space="PSUM") as avsum,
            tc.tile_pool(name="ops", bufs=1, space="PSUM") as opsum,
        ):
            ones = persist.tile([P, 16], BF, tag="ones", name="ones")
            nc.vector.memset(ones[:], 1.0)
            zbias = persist.tile([P, 1], F32, tag="zbias", name="zbias")
            nc.vector.memset(zbias[:], 0.0)
            mk_t = persist.tile([P, 16], BF, tag="mk", name="mk")
            nc.gpsimd.dma_start(out=mk_t[:], in_=msk[:, :])
            qT = [persist.tile([P, ROWS], BF, tag=f"qT{t}", name=f"qT{t}") for t in range(DC)]
            acc = {}
            for r in range(4):
                for ob in range(2):
                    acc[r, ob] = persist.tile([P, 512], F32, tag=f"acc{r}{ob}", name=f"acc{r}{ob}")
                    nc.vector.memset(acc[r, ob][:], 0.0)
            sums_bank = opsum.tile([P, 64], F32, tag="sums", name="sums")

            wk_t = [kp.tile([P, D], BF, tag=f"wk{d_}", name=f"wk{d_}") for d_ in range(DC)]
            wv_t = [kp.tile([P, D], BF, tag=f"wv{d_}", name=f"wv{d_}") for d_ in range(DC)]
            wq_t = [kp.tile([P, D], BF, tag=f"wq{d_}", name=f"wq{d_}") for d_ in range(DC)]
            xk_t = [kp.tile([P, ROWS], BF, tag=f"xk{d_}", name=f"xk{d_}") for d_ in range(DC)]
            xv_t = [kp.tile([P, ROWS], BF, tag=f"xv{d_}", name=f"xv{d_}") for d_ in range(DC)]
            xq_t = [kp.tile([P, ROWS], BF, tag=f"xq{d_}", name=f"xq{d_}") for d_ in range(DC)]

            for d_ in range(DC):
                nc.sync.dma_start(out=wk_t[d_][:], in_=wk[d_ * P:(d_ + 1) * P, :])
                nc.sync.dma_start(out=xk_t[d_][:], in_=xk[d_ * P:(d_ + 1) * P, :])
            for d_ in range(DC):
                nc.sync.dma_start(out=wv_t[d_][:], in_=wv[d_ * P:(d_ + 1) * P, :])
                nc.sync.dma_start(out=xv_t[d_][:], in_=xv[d_ * P:(d_ + 1) * P, :])
            for d_ in range(DC):
                nc.sync.dma_start(out=wq_t[d_][:], in_=wq[d_ * P:(d_ + 1) * P, :])
                nc.sync.dma_start(out=xq_t[d_][:], in_=xq[d_ * P:(d_ + 1) * P, :])

            # ---- K projection: K^T chunk [1024, 512] for own positions ----
            for oc in range(DC):
                ps = pps.tile([P, 512], F32, tag="pp", name="ppk")
                for d_ in range(DC):
                    nc.tensor.matmul(
                        ps[:],
                        lhsT=wk_t[d_][:, oc * P:(oc + 1) * P],
                        rhs=xk_t[d_][:],
                        start=(d_ == 0),
                        stop=(d_ == DC - 1),
                    )
                sg = stg.tile([P, 512], BF, tag="sg", name="sgk")
                nc.scalar.copy(sg[:], ps[:])
                nc.gpsimd.dma_start(out=kvin[0][:, 256 * oc:256 * oc + 256], in_=sg[:, 0:256])
                nc.gpsimd.dma_start(out=kvin[1][:, 256 * oc:256 * oc + 256], in_=sg[:, 256:512])

            # ---- V projection: V chunk [512, 1024], by position tile jh ----
            for jh in range(4):
                for ob in range(2):
                    ps = pps.tile([P, 512], F32, tag="pp", name="ppv")
                    for d_ in range(DC):
                        nc.tensor.matmul(
                            ps[:],
                            lhsT=xv_t[d_][:, jh * P:(jh + 1) * P],
                            rhs=wv_t[d_][:, ob * 512:(ob + 1) * 512],
                            start=(d_ == 0),
                            stop=(d_ == DC - 1),
                        )
                    sg = stg.tile([P, 512], BF, tag="sg", name="sgv")
                    nc.scalar.copy(sg[:], ps[:])
                    h = jh // 2
                    off = 2048 + 1024 * (jh % 2) + 512 * ob
                    nc.gpsimd.dma_start(out=kvin[h][:, off:off + 512], in_=sg[:])
                if jh == 1:
                    nc.gpsimd.collective_compute(
                        "AllGather",
                        mybir.AluOpType.bypass,
                        replica_groups=ALL8,
                        ins=[kvin[0][:].opt()],
                        outs=[kvout[0][:].opt()],
                    )
            nc.gpsimd.collective_compute(
                "AllGather",
                mybir.AluOpType.bypass,
                replica_groups=ALL8,
                ins=[kvin[1][:].opt()],
                outs=[kvout[1][:].opt()],
            )

            # ---- Q projection -> qT (bf16, packed rows) ----
            for oc in range(DC):
                ps = pps.tile([P, 512], F32, tag="pp", name="ppq")
                for d_ in range(DC):
                    nc.tensor.matmul(
                        ps[:],
                        lhsT=wq_t[d_][:, oc * P:(oc + 1) * P],
                        rhs=xq_t[d_][:],
                        start=(d_ == 0),
                        stop=(d_ == DC - 1),
                    )
                nc.vector.tensor_copy(qT[oc][:], ps[:])

            # ---- attention: stream gathered chunk-halves ----
            def nrows(j):
                return 512 - 16 * j

            for h in range(2):
                for m in range(NCORES):
                    kv = kvs.tile([P, 4096], BF, tag="kv", name="kv")
                    nc.scalar.dma_start(out=kv[:], in_=kvout[h][m * P:(m + 1) * P, :])
                    pts = []
                    for jl in range(2):
                        j = 4 * m + 2 * h + jl
                        N = nrows(j)
                        sp = pps.tile([P, 512], F32, tag="pp", name="sps")
                        for oc in range(DC):
                            nc.tensor.matmul(
                                sp[:, 0:N],
                                lhsT=kv[:, 256 * oc + 128 * jl:256 * oc + 128 * jl + 128],
                                rhs=qT[oc][:, 0:N],
                                start=(oc == 0),
                                stop=(oc == DC - 1),
                            )
                        pt = ap.tile([P, 512], BF, tag="pt", name="pt")
                        nc.scalar.activation(pt[:, 0:N], sp[:, 0:N], EXP, bias=zbias[:])
                        nc.vector.tensor_mul(pt[:, N - 16:N], pt[:, N - 16:N], mk_t[:])
                        pts.append((pt, N, jl))
                        for r in range((N + 127) // 128):
                            M = min(128, N - 128 * r)
                            nc.tensor.matmul(
                                sums_bank[0:M, 16 * r:16 * r + 16],
                                lhsT=pt[:, 128 * r:128 * r + M],
                                rhs=ones[:],
                                start=(j == 0),
                                stop=(j == 31 - 8 * r),
                                skip_group_check=True,
                            )
                    # A@V for this chunk-half, accumulated over its 2 tiles
                    N0 = pts[0][1]
                    for r in range((N0 + 127) // 128):
                        live = [(pt, min(128, N - 128 * r), jl)
                                for (pt, N, jl) in pts if N > 128 * r]
                        M0 = live[0][1]
                        for ob in range(2):
                            sc = avsum.tile([P, 512], F32, tag="avs", name="avs")
                            for i, (pt, M, jl) in enumerate(live):
                                nc.tensor.matmul(
                                    sc[0:M, :],
                                    lhsT=pt[:, 128 * r:128 * r + M],
                                    rhs=kv[:, 2048 + 1024 * jl + 512 * ob:
                                           2048 + 1024 * jl + 512 * ob + 512],
                                    start=(i == 0),
                                    stop=(i == len(live) - 1),
                                )
                            nc.vector.tensor_add(
                                acc[r, ob][0:M0, :], acc[r, ob][0:M0, :], sc[0:M0, :]
                            )

            # ---- normalize and write out ----
            for r in range(4):
                ssb = op.tile([P, 1], F32, tag="ssb", name="ssb")
                nc.vector.tensor_copy(ssb[:], sums_bank[:, 16 * r:16 * r + 1])
                rec = op.tile([P, 1], F32, tag=f"rec{r}", name=f"rec{r}")
                nc.vector.reciprocal(rec[:], ssb[:])
                for ob in range(2):
                    osb = op.tile([P, 512], F32, tag="osb", name="osb")
                    nc.vector.tensor_scalar_mul(osb[:], acc[r, ob][:], rec[:])
                    nc.sync.dma_start(
                        out=out[r * P:(r + 1) * P, ob * 512:(ob + 1) * 512], in_=osb[:]
                    )
    return nc


_CACHE = {}


def _get_nc():
    if "nc" not in _CACHE:
        nc = build_nc()
        nc.compile()
        _CACHE["nc"] = nc
    return _CACHE["nc"]


def _rows_desc(c):
    return sorted([r for r in range(S) if (r // 16) % NCORES == c], reverse=True)


def build_in_maps(inputs):
    x_q = np.asarray(inputs["encodings_for_q"], dtype=np.float32)
    x_k = np.asarray(inputs["encodings_for_k"], dtype=np.float32)
    x_v = np.asarray(inputs["encodings_for_v"], dtype=np.float32)
    W_q = np.asarray(inputs["W_q"], dtype=np.float32)
    W_k = np.asarray(inputs["W_k"], dtype=np.float32)
    W_v = np.asarray(inputs["W_v"], dtype=np.float32)

    wqt = np.ascontiguousarray(W_q.T).astype(bf16)
    wkt = np.ascontiguousarray(W_k.T / np.sqrt(D)).astype(bf16)
    wvt = np.ascontiguousarray(W_v.T).astype(bf16)

    p_idx = np.arange(P)[:, None]
    t_idx = np.arange(16)[None, :]

    in_maps = []
    for c in range(NCORES):
        rows = _rows_desc(c)
        pos = slice(ROWS * c, ROWS * (c + 1))
        xqt_c = np.ascontiguousarray(x_q[rows].T).astype(bf16)
        xkt_c = np.ascontiguousarray(x_k[pos].T).astype(bf16)
        xvt_c = np.ascontiguousarray(x_v[pos].T).astype(bf16)
        mk_c = (p_idx <= 16 * c + 15 - t_idx).astype(bf16)
        in_maps.append(
            dict(
                xqt=xqt_c, xkt=xkt_c, xvt=xvt_c,
                wqt=wqt, wkt=wkt, wvt=wvt,
                mk=np.ascontiguousarray(mk_c),
            )
        )
    return in_maps


def kernel(**inputs):
    nc = _get_nc()
    in_maps = build_in_maps(inputs)
    res = run_bass_kernel_spmd(nc, in_maps, list(range(NCORES)))
    full = np.zeros((S, D), dtype=np.float32)
    for c in range(NCORES):
        full[_rows_desc(c)] = np.asarray(res.results[c]["out"], dtype=np.float32)
    return full
